# revision 37
# baseline (speedup 1.0000x reference)
"""GaussianImage splat kernel for 8 Trainium2 NeuronCores.

Math: for gaussian n with mean mu, covariance C = R S S^T R^T,
  prob[n, p] = exp(z[n, p]),   z = -0.5 * (d^T C^-1 d) - log(2*pi*sqrt(det C))
  img = sigmoid( (prob / max(prob)) @ (rgb * alpha) )

z is a quadratic polynomial in the pixel coords (x, y): per gaussian a dot
product of per-gaussian coefficients with features [x^2, xy, y^2, x, y] plus
a constant (folded into the exp bias).  On device, z for 128 gaussians x 512
pixels is one K=5 matmul; four such run concurrently in the PE's four row
groups (pixel chunks 4t..4t+3).  exp runs on the scalar engine over two
[128, 1024] halves.  The sum over gaussians weighted by rgb*alpha is a K=128
bf16 matmul accumulated in PSUM; the four 512-pixel chunks ride in the four
PE column groups of a single PSUM bank, ordered behind a full-width
bank-clearing dummy matmul.

Precision: gaussians are norm-sorted, so chunk 0 holds every ill-conditioned
(large-coefficient) gaussian and uses fp32 z matmuls; chunks 1-3 use
accumulating bf16 matmuls with hi/lo-split operands (PASSES per chunk, error
bound host-checked post-run against the measured max, numpy fallback).

Sharding: image rows are sharded across the 8 cores; all 512 gaussians are
resident per core, so no all-reduce(sum) is needed.  The global max: since
prob <= amplitude = 1/(2*pi*sqrt(det)), the max provably lies in the top-128
chunk whenever (max over that chunk) >= (129th amplitude) — host-verified.
A max-only prepass (fp32 z, reduce on PSUM, no exp) computes the local max
early so the ~70us all-reduce(max) collective overlaps the main loop.
"""

import os
import numpy as np

N = 512          # gaussians
H = 512
W = 512
NCORES = 8
ROWS_PER_CORE = H // NCORES          # 64
TQ = 16                              # pair iterations per core
Q = 4                                # 512-px chunks per pair (PE groups)
PW = 512                             # pixels per chunk (matmul free dim)
G = 4                                # gaussian chunks of 128
PASSES = [0, 3, 2, 2]                # z passes per chunk (0 = fp32 matmul)

_CACHE: dict = {}
LAST_RESULTS = None  # BassKernelResults of the most recent device run


def _ensure_axon_hooks():
    """Register an antenv.axon_hooks shim if the image lacks one, so
    run_bass_kernel_spmd(trace=True) can reach NRT profiling via the axon
    PJRT library instead of crashing on the import."""
    try:
        import antenv.axon_hooks  # noqa: F401
        return
    except ImportError:
        pass
    import sys
    import types
    import ctypes
    import contextlib
    try:
        import antenv
    except ImportError:
        return
    mod = types.ModuleType("antenv.axon_hooks")
    holder = {"hook": None}
    mod.set_axon_ntff_profile_hook = lambda h: holder.__setitem__("hook", h)
    mod.get_axon_ntff_profile_hook = lambda: holder["hook"]
    sys.modules["antenv.axon_hooks"] = mod
    antenv.axon_hooks = mod

    so_path = "/opt/axon/libaxon_pjrt.so"
    if not os.path.exists(so_path):
        return
    try:
        lib = ctypes.CDLL(so_path)
        if not hasattr(lib, "axon_start_nrt_profile"):
            return
        lib.axon_start_nrt_profile.argtypes = [
            ctypes.POINTER(ctypes.c_int64), ctypes.c_size_t]
        lib.axon_start_nrt_profile.restype = ctypes.c_int64
        lib.axon_stop_nrt_profile.argtypes = [ctypes.c_char_p]
        lib.axon_stop_nrt_profile.restype = ctypes.c_int64

        @contextlib.contextmanager
        def _hook(output_dir, device_ids):
            import jax
            jax.devices()
            if device_ids:
                ids = (ctypes.c_int64 * len(device_ids))(*device_ids)
                rc = lib.axon_start_nrt_profile(ids, len(device_ids))
            else:
                rc = lib.axon_start_nrt_profile(None, 0)
            if rc != 0:
                raise RuntimeError(f"axon_start_nrt_profile rc={rc}")
            try:
                yield
            finally:
                n = lib.axon_stop_nrt_profile(str(output_dir).encode())
                print(f"profile: {n} file(s) written to {output_dir}")

        mod.set_axon_ntff_profile_hook(_hook)
    except Exception:
        pass


def _build():
    import concourse.bass as bass  # noqa: F401
    import concourse.bacc as bacc
    import concourse.tile as tile
    from concourse import mybir
    import concourse.bass_isa as bass_isa

    f32 = mybir.dt.float32
    b16 = mybir.dt.bfloat16
    nc = bacc.Bacc(None, target_bir_lowering=False, debug=False,
                   num_devices=NCORES)

    # stage row k' = TQ*q + t holds pixel chunk k = Q*t + q (host permutes)
    xk = nc.dram_tensor("xk", [Q * TQ, PW], f32, kind="ExternalInput").ap()
    yk = nc.dram_tensor("yk", [Q * TQ, PW], f32, kind="ExternalInput").ap()
    coef = nc.dram_tensor("coef", [5, N], f32, kind="ExternalInput").ap()
    coefh = nc.dram_tensor("coefh", [5, N], b16, kind="ExternalInput").ap()
    coefl = nc.dram_tensor("coefl", [5, N], b16, kind="ExternalInput").ap()
    cbias = nc.dram_tensor("cbias", [128, G], f32, kind="ExternalInput").ap()
    wrgb = nc.dram_tensor("wrgb", [N, 3], b16, kind="ExternalInput").ap()
    dzw = nc.dram_tensor("dzw", [1, 128], b16, kind="ExternalInput").ap()
    done_ = nc.dram_tensor("done", [1, PW], b16, kind="ExternalInput").ap()
    # img row 3*(2t+m)+ch, free 512*r+w  (m = member, r = chunk in member)
    img = nc.dram_tensor("img", [3 * 2 * TQ, 2 * PW], f32,
                         kind="ExternalOutput").ap()
    omax = nc.dram_tensor("omax", [1, 1], f32, kind="ExternalOutput").ap()

    with tile.TileContext(nc) as tc:
        with (
            tc.tile_pool(name="const", bufs=1) as cpool,
            tc.tile_pool(name="prob", bufs=4) as ppool,
            tc.tile_pool(name="tmp", bufs=2) as tpool,
            tc.tile_pool(name="zp", bufs=3, space="PSUM") as zpool,
            tc.tile_pool(name="accp", bufs=2, space="PSUM") as apool,
            tc.tile_pool(name="dram", bufs=1, space="DRAM") as dpool,
        ):
            phi = cpool.tile([128, TQ * PW], f32)      # pixel features fp32
            phih = cpool.tile([128, TQ * PW], b16)     # features hi (bf16)
            phil = cpool.tile([128, TQ * PW], b16)     # features lo residual
            c6 = cpool.tile([128, N], f32)             # z-coefs (Q replicas)
            c6h = cpool.tile([128, N], b16)
            c6l = cpool.tile([128, N], b16)
            cb = cpool.tile([128, G], f32)             # z const term (exp bias)
            wt = cpool.tile([128, G * 3], b16)         # rgb*alpha per chunk
            dz = cpool.tile([1, 128], b16)             # dummy-clear weights (0)
            do_ = cpool.tile([1, PW], b16)             # dummy-clear rhs (1)
            stash = cpool.tile([6 * TQ, 2 * PW], f32)  # unnormalized image
            sig = cpool.tile([6 * TQ, 2 * PW], f32)    # final image
            mxc = cpool.tile([128, 2 * TQ], f32)       # per-half-pair maxes
            mxf = cpool.tile([128, 1], f32)
            pm = cpool.tile([128, 1], f32)
            pmb = cpool.tile([128, 1], f32)
            gmx = cpool.tile([128, 1], f32)
            inv = cpool.tile([128, 1], f32)

            # phi layout: partition 32*q + f holds feature f of pixel chunk
            # (Q*t + q); free offset 512*t + w.  f in [x2, xy, y2, x, y].
            # DVE lanes are partition-hardwired and compute engines need unit
            # partition steps, so features are built in dense [64, 512] stage
            # tiles, then DMA-scattered into the strided phi layout (DMA
            # remaps partitions; device-computed values bounce through DRAM
            # scratch since SBUF-to-SBUF partition scatter is not one AP).
            xs = cpool.tile([Q * TQ, PW], f32)
            ys = cpool.tile([Q * TQ, PW], f32)
            x2 = cpool.tile([Q * TQ, PW], f32)
            xy = cpool.tile([Q * TQ, PW], f32)
            y2 = cpool.tile([Q * TQ, PW], f32)
            fhi = [cpool.tile([Q * TQ, PW], b16, name=f"fhi{i}", tag=f"fhi{i}")
                   for i in range(5)]
            flo = [cpool.tile([Q * TQ, PW], b16, name=f"flo{i}", tag=f"flo{i}")
                   for i in range(5)]
            phi_v = phi[:].rearrange("(q f) (t w) -> q f t w", f=32, w=PW)
            phih_v = phih[:].rearrange("(q f) (t w) -> q f t w", f=32, w=PW)
            phil_v = phil[:].rearrange("(q f) (t w) -> q f t w", f=32, w=PW)

            nc.sync.dma_start(xs[:], xk[:])
            nc.sync.dma_start(ys[:], yk[:])

            c6_v = c6[:].rearrange("(q f) j -> q f j", f=32)
            c6h_v = c6h[:].rearrange("(q f) j -> q f j", f=32)
            c6l_v = c6l[:].rearrange("(q f) j -> q f j", f=32)
            for q in range(Q):
                nc.sync.dma_start(c6_v[q, 0:5], coef[:])
                nc.sync.dma_start(c6h_v[q, 0:5], coefh[:])
                nc.sync.dma_start(c6l_v[q, 0:5], coefl[:])
            nc.sync.dma_start(cb[:], cbias)
            wt_v = wt[:].rearrange("j (g c) -> j g c", c=3)
            nc.sync.dma_start(wt_v[:], wrgb.rearrange("(g j) c -> j g c", j=128))
            nc.sync.dma_start(dz[:], dzw)
            nc.sync.dma_start(do_[:], done_)

            mult = mybir.AluOpType.mult
            sub = mybir.AluOpType.subtract
            nc.vector.tensor_tensor(x2[:], xs[:], xs[:], mult)
            nc.vector.tensor_tensor(xy[:], xs[:], ys[:], mult)
            nc.vector.tensor_tensor(y2[:], ys[:], ys[:], mult)
            nc.sync.dma_start(phi_v[:, 3], xk.rearrange("(q t) w -> q t w", t=TQ))
            nc.sync.dma_start(phi_v[:, 4], yk.rearrange("(q t) w -> q t w", t=TQ))
            for f, t in enumerate([x2, xy, y2]):
                scr = dpool.tile([Q * TQ, PW], f32, name=f"scr{f}",
                                 tag=f"scr{f}")
                nc.sync.dma_start(scr[:], t[:])
                nc.sync.dma_start(
                    phi_v[:, f],
                    scr[:].rearrange("(q t) w -> q t w", t=TQ))

            exp_f = mybir.ActivationFunctionType.Exp

            def z_matmuls(zts, t, g):
                """Emit the four row-group-packed z matmuls for (t, g).
                zts = (ztA, ztB): [128, 1024] PSUM tiles for members 0/1."""
                gsl = slice(128 * g, 128 * (g + 1))
                np_ = PASSES[g]
                for m in range(2):
                    for r in range(2):
                        q = 2 * m + r
                        zsl = zts[m][:, PW * r:PW * (r + 1)]
                        if np_ == 0:
                            nc.tensor.matmul(
                                zsl, c6_v[q, 0:5, gsl], phi_v[q, 0:5, t],
                                start=True, stop=True,
                                tile_position=(32 * q, 0))
                        else:
                            ops = [(c6h_v, phih_v), (c6h_v, phil_v),
                                   (c6l_v, phih_v)][:np_]
                            for p, (cc, pp) in enumerate(ops):
                                nc.tensor.matmul(
                                    zsl, cc[q, 0:5, gsl], pp[q, 0:5, t],
                                    start=(p == 0), stop=(p == np_ - 1),
                                    tile_position=(32 * q, 0))

            def prepass_step(t):
                # max-only step over chunk 0 (fp32 z, reduce on PSUM, no
                # exp); interleaved into the early main loop so the zt/acc
                # buffer rotation never serializes behind it.
                ztA = zpool.tile([128, 2 * PW], f32, name="zt", tag="zt")
                ztB = zpool.tile([128, 2 * PW], f32, name="zt", tag="zt")
                z_matmuls((ztA, ztB), t, 0)
                nc.vector.reduce_max(mxc[:, 2 * t:2 * t + 1], ztA[:],
                                     mybir.AxisListType.X)
                nc.vector.reduce_max(mxc[:, 2 * t + 1:2 * t + 2], ztB[:],
                                     mybir.AxisListType.X)

            # bf16 feature splits (deferred here so their DVE/DMA work does
            # not gate the prepass; only needed from the main loop's g=1 on)
            feats = [x2, xy, y2, xs, ys]
            for i, t in enumerate(feats):
                nc.vector.tensor_copy(fhi[i][:], t[:])           # bf16 cast
                nc.vector.tensor_tensor(flo[i][:], t[:], fhi[i][:], sub)
            for f in range(5):
                for which, stage, dst in (("h", fhi, phih_v), ("l", flo, phil_v)):
                    scr = dpool.tile([Q * TQ, PW], b16, name=f"scr{which}{f}",
                                     tag=f"scr{which}{f}")
                    nc.sync.dma_start(scr[:], stage[f][:])
                    nc.sync.dma_start(
                        dst[:, f],
                        scr[:].rearrange("(q t) w -> q t w", t=TQ))

            def emit_collective():
                # per-gaussian max prob = exp(max z + const).  All bounce
                # DMAs ride the gpsimd (SWDGE) queue: the post-collective
                # load blocks its queue for the collective's latency, which
                # must not stall the sync-queue eviction DMAs.
                nc.vector.reduce_max(mxf[:], mxc[:], mybir.AxisListType.X)
                nc.scalar.activation(pm[:], mxf[:], exp_f, bias=cb[:, 0:1])
                nc.gpsimd.partition_all_reduce(pmb[:], pm[:], channels=128,
                                               reduce_op=bass_isa.ReduceOp.max)
                cin = dpool.tile([128, 1], f32, name="cin", tag="cin")
                cout = dpool.tile([128, 1], f32, name="cout", tag="cout")
                nc.gpsimd.dma_start(cin[:], pmb[:])
                nc.gpsimd.collective_compute(
                    "AllReduce", mybir.AluOpType.max,
                    replica_groups=[list(range(NCORES))],
                    ins=[cin.opt()], outs=[cout.opt()],
                )
                nc.gpsimd.dma_start(gmx[:], cout[:])
                nc.vector.reciprocal(inv[:], gmx[:])

            # ---- main loop.  The acc matmuls for step (t, g) are emitted
            # AFTER the z matmuls of the next step: the PE queue is strict
            # FIFO, so emitting them right after their exp producer would
            # stall PE waiting on the activation.  Deferring one step keeps
            # PE dense; exp(t, g) completes during the next z block.
            pending = None  # (probA, probB, g, t)
            accs = {}       # t -> acc psum tile

            def flush_pending():
                pA, pB, g_, t_ = pending
                if g_ == 0:
                    acc_ = apool.tile([128, PW], f32, name="acc", tag="acc")
                    accs[t_] = acc_
                    # full-width dummy matmul: start=True clears the whole
                    # acc bank; its full-array occupancy orders its (zero)
                    # writes before the col-tiled acc matmuls below.
                    nc.tensor.matmul(acc_[:, :], dz[0:1, :], do_[0:1, :],
                                     start=True, stop=False,
                                     skip_group_check=True)
                acc_ = accs[t_]
                for m, pr in ((0, pA), (1, pB)):
                    for r in range(2):
                        q = 2 * m + r
                        # acc[32q:32q+3] += wt^T @ prob; four chunks ride in
                        # the four PE column groups of one PSUM bank.
                        nc.tensor.matmul(
                            acc_[32 * q:32 * q + 3, :],
                            wt_v[:, g_],
                            pr[:, PW * r:PW * (r + 1)],
                            start=False, stop=(g_ == G - 1),
                            tile_position=(0, 32 * q),
                            skip_group_check=True,
                        )
                if g_ == G - 1:
                    tmp = tpool.tile([128, PW], f32, name="tmp", tag="tmp")
                    nc.vector.tensor_copy(tmp[:], acc_[:])
                    # pack into dense partitions for the final sigmoid
                    for m in range(2):
                        for r in range(2):
                            q = 2 * m + r
                            srow = 3 * (2 * t_ + m)
                            nc.sync.dma_start(
                                stash[srow:srow + 3, PW * r:PW * (r + 1)],
                                tmp[32 * q:32 * q + 3, :])
                    del accs[t_]

            for t in range(TQ):
                for g in range(G):
                    if t < 8 and g in (0, 2):
                        prepass_step(2 * t + g // 2)
                    ztA = zpool.tile([128, 2 * PW], f32, name="zt", tag="zt")
                    ztB = zpool.tile([128, 2 * PW], f32, name="zt", tag="zt")
                    z_matmuls((ztA, ztB), t, g)
                    if pending is not None:
                        flush_pending()
                    probA = ppool.tile([128, 2 * PW], b16, name="probA",
                                       tag="prob")
                    probB = ppool.tile([128, 2 * PW], b16, name="probB",
                                       tag="prob")
                    nc.scalar.activation(probA[:], ztA[:], exp_f,
                                         bias=cb[:, g:g + 1])
                    nc.scalar.activation(probB[:], ztB[:], exp_f,
                                         bias=cb[:, g:g + 1])
                    pending = (probA, probB, g, t)
                if t == 7:
                    emit_collective()
            flush_pending()

            nc.scalar.activation(sig[:], stash[:],
                                 mybir.ActivationFunctionType.Sigmoid,
                                 scale=inv[0:6 * TQ, 0:1])
            nc.sync.dma_start(img, sig[:])
            nc.sync.dma_start(omax, gmx[0:1, 0:1])

    nc.compile()
    return nc


def _get_nc():
    if "nc" not in _CACHE:
        _CACHE["nc"] = _build()
    return _CACHE["nc"]


def _params(mean, alpha, scale, theta, rgb):
    """Per-gaussian z-polynomial coefficients (float64 host math)."""
    ta = 2.0 * np.pi * theta[:, 0].astype(np.float64)
    c, s = np.cos(ta), np.sin(ta)
    sx2 = scale[:, 0].astype(np.float64) ** 2
    sy2 = scale[:, 1].astype(np.float64) ** 2
    a = c * c * sx2 + s * s * sy2
    b = c * s * (sx2 - sy2)
    d = s * s * sx2 + c * c * sy2
    det = a * d - b * b
    inv00 = d / det
    inv11 = a / det
    invc = -2.0 * b / det                 # coefficient of dx*dy in q
    lognorm = -np.log(2.0 * np.pi * np.sqrt(det))
    mx = mean[:, 0, 0].astype(np.float64)
    my = mean[:, 1, 0].astype(np.float64)
    c1 = -0.5 * inv00
    c2 = -0.5 * invc
    c3 = -0.5 * inv11
    c4 = inv00 * mx + 0.5 * invc * my
    c5 = 0.5 * invc * mx + inv11 * my
    c6 = -0.5 * (inv00 * mx * mx + invc * mx * my + inv11 * my * my) + lognorm
    C = np.stack([c1, c2, c3, c4, c5, c6], 0)          # (6, N) float64
    wt = rgb.astype(np.float64) * alpha.astype(np.float64)  # (N, 3)
    norm = np.exp(lognorm)
    return C, wt, norm


def _numpy_kernel(mean, alpha, scale, theta, rgb, pixels):
    """Exact host fallback (used only if a correctness guard fails)."""
    C, wt, norm = _params(mean, alpha, scale, theta, rgb)
    x = pixels[:, :, 0].reshape(-1).astype(np.float64)
    y = pixels[:, :, 1].reshape(-1).astype(np.float64)
    Phi = np.stack([x * x, x * y, y * y, x, y, np.ones_like(x)], 0)
    acc = np.zeros((x.size, 3), dtype=np.float64)
    pmax = 0.0
    for i in range(0, N, 64):
        prob = np.exp(C[:, i:i + 64].T @ Phi)
        pmax = max(pmax, prob.max())
        acc += prob.T @ wt[i:i + 64]
    out = 1.0 / (1.0 + np.exp(-acc / pmax))
    return out.reshape(H, W, 3).astype(np.float32)


def kernel(mean, alpha, scale, theta, rgb, pixels):
    global LAST_RESULTS
    mean = np.asarray(mean, dtype=np.float32)
    alpha = np.asarray(alpha, dtype=np.float32)
    scale = np.asarray(scale, dtype=np.float32)
    theta = np.asarray(theta, dtype=np.float32)
    rgb = np.asarray(rgb, dtype=np.float32)
    pixels = np.asarray(pixels, dtype=np.float32)

    import ml_dtypes
    bf16 = ml_dtypes.bfloat16

    C, wt, norm = _params(mean, alpha, scale, theta, rgb)
    order = np.argsort(-norm, kind="stable")
    norms = norm[order]
    Cs = np.ascontiguousarray(C[0:5, order]).astype(np.float32)
    Ch = Cs.astype(bf16)
    Cl = (Cs.astype(np.float64) - Ch.astype(np.float64)).astype(
        np.float32).astype(bf16)
    cbias = np.ascontiguousarray(
        C[5, order].reshape(G, 128).T).astype(np.float32)      # [128, G]
    wts = np.ascontiguousarray(wt[order]).astype(np.float32)
    nu = float(norms[128])               # largest amplitude outside chunk 0

    # stage row k' = TQ*q + t holds pixel chunk k = Q*t + q of the core slab
    perm = np.array([Q * (kp % TQ) + kp // TQ for kp in range(Q * TQ)])
    in_maps = []
    for c in range(NCORES):
        slab = pixels[ROWS_PER_CORE * c:ROWS_PER_CORE * (c + 1)]  # (64,512,2)
        in_maps.append({
            "xk": np.ascontiguousarray(slab[:, :, 0].reshape(Q * TQ, PW)[perm]),
            "yk": np.ascontiguousarray(slab[:, :, 1].reshape(Q * TQ, PW)[perm]),
            "coef": Cs,
            "coefh": Ch,
            "coefl": Cl,
            "cbias": cbias,
            "wrgb": wts.astype(bf16),
            "dzw": np.zeros((1, 128), dtype=bf16),
            "done": np.ones((1, PW), dtype=bf16),
        })

    from concourse import bass_utils
    nc = _get_nc()
    want_trace = bool(int(os.environ.get("GAUSS_TRACE", "0")))
    if want_trace:
        _ensure_axon_hooks()
    try:
        res = bass_utils.run_bass_kernel_spmd(
            nc, in_maps, core_ids=list(range(NCORES)), trace=want_trace,
        )
    except Exception:
        if not want_trace:
            raise
        res = bass_utils.run_bass_kernel_spmd(
            nc, in_maps, core_ids=list(range(NCORES)), trace=False,
        )
    LAST_RESULTS = res

    out_slabs = []
    for c in range(NCORES):
        a = res.results[c]["img"].reshape(2 * TQ, 3, 2, PW)   # (s, ch, r, w)
        a = np.transpose(a, (0, 2, 3, 1)).reshape(ROWS_PER_CORE, W, 3)
        out_slabs.append(a)
    out = np.concatenate(out_slabs, axis=0)
    m0 = float(res.results[0]["omax"][0, 0])

    ok = np.isfinite(m0) and m0 >= 1.05 * nu and bool(np.all(np.isfinite(out)))
    if ok:
        # split-bf16 z error must be negligible after max-normalization:
        # chunk g's splat weight is <= norms[128g] / m0.
        phimax = max(1.0, float(np.abs(pixels).max()) ** 2)
        delta = {3: 2.0 ** -16, 2: 2.0 ** -8}
        for g in range(1, G):
            sc = float(np.abs(Cs[:, 128 * g:128 * (g + 1)]).sum(0).max())
            zerr = sc * phimax * delta[PASSES[g]]
            img_err = (np.expm1(min(zerr, 30.0))) * norms[128 * g] / m0
            if img_err > 2e-3:
                ok = False
                break
    if not ok:
        return _numpy_kernel(mean, alpha, scale, theta, rgb, pixels)
    return out.astype(np.float32)


# revision 39
# speedup vs baseline: 1.1903x; 1.1903x over previous
"""GaussianImage splat kernel for 8 Trainium2 NeuronCores.

Math: for gaussian n with mean mu, covariance C = R S S^T R^T,
  prob[n, p] = exp(z[n, p]),   z = -0.5 * (d^T C^-1 d) - log(2*pi*sqrt(det C))
  img = sigmoid( (prob / max(prob)) @ (rgb * alpha) )

z is a quadratic polynomial in the pixel coords (x, y): per gaussian a dot
product of per-gaussian coefficients with features [x^2, xy, y^2, x, y] plus
a constant (folded into the exp bias).  On device, z for 128 gaussians x 512
pixels is one K=5 matmul; four such run concurrently in the PE's four row
groups (pixel chunks 4t..4t+3).  exp runs on the scalar engine over two
[128, 1024] halves.  The sum over gaussians weighted by rgb*alpha is a K=128
bf16 matmul accumulated in PSUM; the four 512-pixel chunks ride in the four
PE column groups of a single PSUM bank, ordered behind a full-width
bank-clearing dummy matmul.

Precision: gaussians are norm-sorted, so chunk 0 holds every ill-conditioned
(large-coefficient) gaussian and uses fp32 z matmuls; chunks 1-3 use
accumulating bf16 matmuls with hi/lo-split operands (PASSES per chunk, error
bound host-checked post-run against the measured max, numpy fallback).

Sharding: image rows are sharded across the 8 cores; all 512 gaussians are
resident per core, so no all-reduce(sum) is needed.  The global max: since
prob <= amplitude = 1/(2*pi*sqrt(det)), the max provably lies in the top-128
chunk whenever (max over that chunk) >= (129th amplitude) — host-verified.
A max-only prepass (fp32 z, reduce on PSUM, no exp) computes the local max
early so the ~70us all-reduce(max) collective overlaps the main loop.
"""

import os
import numpy as np

N = 512          # gaussians
H = 512
W = 512
NCORES = 8
ROWS_PER_CORE = H // NCORES          # 64
TQ = 16                              # pair iterations per core
Q = 4                                # 512-px chunks per pair (PE groups)
PW = 512                             # pixels per chunk (matmul free dim)
G = 4                                # gaussian chunks of 128
PASSES = [0, 3, 2, 2]                # z passes per chunk (0 = fp32 matmul)

_CACHE: dict = {}
LAST_RESULTS = None  # BassKernelResults of the most recent device run


def _ensure_axon_hooks():
    """Register an antenv.axon_hooks shim if the image lacks one, so
    run_bass_kernel_spmd(trace=True) can reach NRT profiling via the axon
    PJRT library instead of crashing on the import."""
    try:
        import antenv.axon_hooks  # noqa: F401
        return
    except ImportError:
        pass
    import sys
    import types
    import ctypes
    import contextlib
    try:
        import antenv
    except ImportError:
        return
    mod = types.ModuleType("antenv.axon_hooks")
    holder = {"hook": None}
    mod.set_axon_ntff_profile_hook = lambda h: holder.__setitem__("hook", h)
    mod.get_axon_ntff_profile_hook = lambda: holder["hook"]
    sys.modules["antenv.axon_hooks"] = mod
    antenv.axon_hooks = mod

    so_path = "/opt/axon/libaxon_pjrt.so"
    if not os.path.exists(so_path):
        return
    try:
        lib = ctypes.CDLL(so_path)
        if not hasattr(lib, "axon_start_nrt_profile"):
            return
        lib.axon_start_nrt_profile.argtypes = [
            ctypes.POINTER(ctypes.c_int64), ctypes.c_size_t]
        lib.axon_start_nrt_profile.restype = ctypes.c_int64
        lib.axon_stop_nrt_profile.argtypes = [ctypes.c_char_p]
        lib.axon_stop_nrt_profile.restype = ctypes.c_int64

        @contextlib.contextmanager
        def _hook(output_dir, device_ids):
            import jax
            jax.devices()
            if device_ids:
                ids = (ctypes.c_int64 * len(device_ids))(*device_ids)
                rc = lib.axon_start_nrt_profile(ids, len(device_ids))
            else:
                rc = lib.axon_start_nrt_profile(None, 0)
            if rc != 0:
                raise RuntimeError(f"axon_start_nrt_profile rc={rc}")
            try:
                yield
            finally:
                n = lib.axon_stop_nrt_profile(str(output_dir).encode())
                print(f"profile: {n} file(s) written to {output_dir}")

        mod.set_axon_ntff_profile_hook(_hook)
    except Exception:
        pass


def _build():
    import concourse.bass as bass  # noqa: F401
    import concourse.bacc as bacc
    import concourse.tile as tile
    from concourse import mybir
    import concourse.bass_isa as bass_isa

    f32 = mybir.dt.float32
    b16 = mybir.dt.bfloat16
    nc = bacc.Bacc(None, target_bir_lowering=False, debug=False,
                   num_devices=NCORES)

    # stage row k' = TQ*q + t holds pixel chunk k = Q*t + q (host permutes)
    xk = nc.dram_tensor("xk", [Q * TQ, PW], f32, kind="ExternalInput").ap()
    yk = nc.dram_tensor("yk", [Q * TQ, PW], f32, kind="ExternalInput").ap()
    coef = nc.dram_tensor("coef", [5, N], f32, kind="ExternalInput").ap()
    coefh = nc.dram_tensor("coefh", [5, N], b16, kind="ExternalInput").ap()
    coefl = nc.dram_tensor("coefl", [5, N], b16, kind="ExternalInput").ap()
    cbias = nc.dram_tensor("cbias", [128, G], f32, kind="ExternalInput").ap()
    wrgb = nc.dram_tensor("wrgb", [N, 3], b16, kind="ExternalInput").ap()
    dzw = nc.dram_tensor("dzw", [1, 128], b16, kind="ExternalInput").ap()
    done_ = nc.dram_tensor("done", [1, PW], b16, kind="ExternalInput").ap()
    # img row 3*(2t+m)+ch, free 512*r+w  (m = member, r = chunk in member)
    img = nc.dram_tensor("img", [3 * 2 * TQ, 2 * PW], f32,
                         kind="ExternalOutput").ap()
    omax = nc.dram_tensor("omax", [1, 1], f32, kind="ExternalOutput").ap()

    with tile.TileContext(nc) as tc:
        with (
            tc.tile_pool(name="const", bufs=1) as cpool,
            tc.tile_pool(name="prob", bufs=4) as ppool,
            tc.tile_pool(name="tmp", bufs=2) as tpool,
            tc.tile_pool(name="zp", bufs=3, space="PSUM") as zpool,
            tc.tile_pool(name="accp", bufs=2, space="PSUM") as apool,
            tc.tile_pool(name="dram", bufs=1, space="DRAM") as dpool,
        ):
            phi = cpool.tile([128, TQ * PW], f32)      # pixel features fp32
            phih = cpool.tile([128, TQ * PW], b16)     # features hi (bf16)
            phil = cpool.tile([128, TQ * PW], b16)     # features lo residual
            c6 = cpool.tile([128, N], f32)             # z-coefs (Q replicas)
            c6h = cpool.tile([128, N], b16)
            c6l = cpool.tile([128, N], b16)
            cb = cpool.tile([128, G], f32)             # z const term (exp bias)
            wt = cpool.tile([128, G * 3], b16)         # rgb*alpha per chunk
            dz = cpool.tile([1, 128], b16)             # dummy-clear weights (0)
            do_ = cpool.tile([1, PW], b16)             # dummy-clear rhs (1)
            stash = cpool.tile([6 * TQ, 2 * PW], f32)  # unnormalized image
            g0s = [cpool.tile([128, PW], f32, name=f"g0s{t}", tag=f"g0s{t}")
                   for t in range(TQ)]               # chunk-0 acc partials
            sig = cpool.tile([6 * TQ, 2 * PW], f32)    # final image
            mxc = cpool.tile([128, 2 * TQ], f32)       # per-half-pair maxes
            mxf = cpool.tile([128, 1], f32)
            pm = cpool.tile([128, 1], f32)
            pmb = cpool.tile([128, 1], f32)
            gmx = cpool.tile([128, 1], f32)
            inv = cpool.tile([128, 1], f32)

            # phi layout: partition 32*q + f holds feature f of pixel chunk
            # (Q*t + q); free offset 512*t + w.  f in [x2, xy, y2, x, y].
            # DVE lanes are partition-hardwired and compute engines need unit
            # partition steps, so features are built in dense [64, 512] stage
            # tiles, then DMA-scattered into the strided phi layout (DMA
            # remaps partitions; device-computed values bounce through DRAM
            # scratch since SBUF-to-SBUF partition scatter is not one AP).
            xs = cpool.tile([Q * TQ, PW], f32)
            ys = cpool.tile([Q * TQ, PW], f32)
            x2 = cpool.tile([Q * TQ, PW], f32)
            xy = cpool.tile([Q * TQ, PW], f32)
            y2 = cpool.tile([Q * TQ, PW], f32)
            fhi = [cpool.tile([Q * TQ, PW], b16, name=f"fhi{i}", tag=f"fhi{i}")
                   for i in range(5)]
            flo = [cpool.tile([Q * TQ, PW], b16, name=f"flo{i}", tag=f"flo{i}")
                   for i in range(5)]
            phi_v = phi[:].rearrange("(q f) (t w) -> q f t w", f=32, w=PW)
            phih_v = phih[:].rearrange("(q f) (t w) -> q f t w", f=32, w=PW)
            phil_v = phil[:].rearrange("(q f) (t w) -> q f t w", f=32, w=PW)

            nc.sync.dma_start(xs[:], xk[:])
            nc.sync.dma_start(ys[:], yk[:])

            c6_v = c6[:].rearrange("(q f) j -> q f j", f=32)
            c6h_v = c6h[:].rearrange("(q f) j -> q f j", f=32)
            c6l_v = c6l[:].rearrange("(q f) j -> q f j", f=32)
            for q in range(Q):
                nc.sync.dma_start(c6_v[q, 0:5], coef[:])
                nc.sync.dma_start(c6h_v[q, 0:5], coefh[:])
                nc.sync.dma_start(c6l_v[q, 0:5], coefl[:])
            nc.sync.dma_start(cb[:], cbias)
            wt_v = wt[:].rearrange("j (g c) -> j g c", c=3)
            nc.sync.dma_start(wt_v[:], wrgb.rearrange("(g j) c -> j g c", j=128))
            nc.sync.dma_start(dz[:], dzw)
            nc.sync.dma_start(do_[:], done_)

            mult = mybir.AluOpType.mult
            sub = mybir.AluOpType.subtract
            nc.vector.tensor_tensor(x2[:], xs[:], xs[:], mult)
            nc.vector.tensor_tensor(xy[:], xs[:], ys[:], mult)
            nc.vector.tensor_tensor(y2[:], ys[:], ys[:], mult)
            nc.sync.dma_start(phi_v[:, 3], xk.rearrange("(q t) w -> q t w", t=TQ))
            nc.sync.dma_start(phi_v[:, 4], yk.rearrange("(q t) w -> q t w", t=TQ))
            for f, t in enumerate([x2, xy, y2]):
                scr = dpool.tile([Q * TQ, PW], f32, name=f"scr{f}",
                                 tag=f"scr{f}")
                nc.sync.dma_start(scr[:], t[:])
                nc.sync.dma_start(
                    phi_v[:, f],
                    scr[:].rearrange("(q t) w -> q t w", t=TQ))

            exp_f = mybir.ActivationFunctionType.Exp

            def z_matmuls(zts, t, g):
                """Emit the four row-group-packed z matmuls for (t, g).
                zts = (ztA, ztB): [128, 1024] PSUM tiles for members 0/1."""
                gsl = slice(128 * g, 128 * (g + 1))
                np_ = PASSES[g]
                for m in range(2):
                    for r in range(2):
                        q = 2 * m + r
                        zsl = zts[m][:, PW * r:PW * (r + 1)]
                        if np_ == 0:
                            nc.tensor.matmul(
                                zsl, c6_v[q, 0:5, gsl], phi_v[q, 0:5, t],
                                start=True, stop=True,
                                tile_position=(32 * q, 0))
                        else:
                            ops = [(c6h_v, phih_v), (c6h_v, phil_v),
                                   (c6l_v, phih_v)][:np_]
                            for p, (cc, pp) in enumerate(ops):
                                nc.tensor.matmul(
                                    zsl, cc[q, 0:5, gsl], pp[q, 0:5, t],
                                    start=(p == 0), stop=(p == np_ - 1),
                                    tile_position=(32 * q, 0))

            # bf16 feature splits (emitted after the fp32 scatters; only
            # needed from phase 2 on)
            feats = [x2, xy, y2, xs, ys]
            for i, t in enumerate(feats):
                nc.vector.tensor_copy(fhi[i][:], t[:])           # bf16 cast
                nc.vector.tensor_tensor(flo[i][:], t[:], fhi[i][:], sub)
            for f in range(5):
                for which, stage, dst in (("h", fhi, phih_v), ("l", flo, phil_v)):
                    scr = dpool.tile([Q * TQ, PW], b16, name=f"scr{which}{f}",
                                     tag=f"scr{which}{f}")
                    nc.sync.dma_start(scr[:], stage[f][:])
                    nc.sync.dma_start(
                        dst[:, f],
                        scr[:].rearrange("(q t) w -> q t w", t=TQ))

            def emit_collective():
                # per-gaussian max prob = exp(max z + const).  All bounce
                # DMAs ride the gpsimd (SWDGE) queue: the post-collective
                # load blocks its queue for the collective's latency, which
                # must not stall the sync-queue eviction DMAs.
                nc.vector.reduce_max(mxf[:], mxc[:], mybir.AxisListType.X)
                nc.scalar.activation(pm[:], mxf[:], exp_f, bias=cb[:, 0:1])
                nc.gpsimd.partition_all_reduce(pmb[:], pm[:], channels=128,
                                               reduce_op=bass_isa.ReduceOp.max)
                cin = dpool.tile([128, 1], f32, name="cin", tag="cin")
                cout = dpool.tile([128, 1], f32, name="cout", tag="cout")
                nc.gpsimd.dma_start(cin[:], pmb[:])
                nc.gpsimd.collective_compute(
                    "AllReduce", mybir.AluOpType.max,
                    replica_groups=[list(range(NCORES))],
                    ins=[cin.opt()], outs=[cout.opt()],
                )
                nc.gpsimd.dma_start(gmx[:], cout[:])
                nc.vector.reciprocal(inv[:], gmx[:])

            # ---- two-phase schedule.  Phase 1 (g=0, fp32 z): reduce the
            # chunk-0 max from PSUM z, exp, and accumulate the chunk-0
            # partial into a per-t SBUF tile.  The collective fires right
            # after phase 1 and overlaps phase 2 (g=1..3, split-bf16 z),
            # whose eviction adds the chunk-0 partial back (lanes align).
            # The acc matmuls for a step are emitted AFTER the z matmuls of
            # the next step: the PE queue is strict FIFO, so emitting them
            # right after their exp producer would stall PE waiting on the
            # activation.
            schedule = [(t, 0) for t in range(TQ)] + \
                       [(t, g) for t in range(TQ) for g in (1, 2, 3)]
            pending = None  # (probA, probB, g, t)
            accs = {}       # t -> acc psum tile

            def dummy_clear(acc_):
                # full-width matmul: start=True clears the whole acc bank;
                # its full-array occupancy orders its (zero) writes before
                # the col-tiled acc matmuls that follow.
                nc.tensor.matmul(acc_[:, :], dz[0:1, :], do_[0:1, :],
                                 start=True, stop=False,
                                 skip_group_check=True)

            def flush_pending():
                pA, pB, g_, t_ = pending
                if g_ in (0, 1):
                    acc_ = apool.tile([128, PW], f32, name="acc", tag="acc")
                    accs[t_] = acc_
                    dummy_clear(acc_)
                acc_ = accs[t_]
                for m, pr in ((0, pA), (1, pB)):
                    for r in range(2):
                        q = 2 * m + r
                        # acc[32q:32q+3] += wt^T @ prob; four chunks ride in
                        # the four PE column groups of one PSUM bank.
                        nc.tensor.matmul(
                            acc_[32 * q:32 * q + 3, :],
                            wt_v[:, g_],
                            pr[:, PW * r:PW * (r + 1)],
                            start=False, stop=(g_ in (0, G - 1)),
                            tile_position=(0, 32 * q),
                            skip_group_check=True,
                        )
                if g_ == 0:
                    # park the chunk-0 partial in SBUF (same partitions)
                    nc.vector.tensor_copy(g0s[t_][:], acc_[:])
                elif g_ == G - 1:
                    tmp = tpool.tile([128, PW], f32, name="tmp", tag="tmp")
                    nc.vector.tensor_tensor(tmp[:], acc_[:], g0s[t_][:],
                                            mybir.AluOpType.add)
                    # pack into dense partitions for the final sigmoid
                    for m in range(2):
                        for r in range(2):
                            q = 2 * m + r
                            srow = 3 * (2 * t_ + m)
                            nc.sync.dma_start(
                                stash[srow:srow + 3, PW * r:PW * (r + 1)],
                                tmp[32 * q:32 * q + 3, :])
                if g_ in (0, G - 1):
                    del accs[t_]

            for idx, (t, g) in enumerate(schedule):
                ztA = zpool.tile([128, 2 * PW], f32, name="zt", tag="zt")
                ztB = zpool.tile([128, 2 * PW], f32, name="zt", tag="zt")
                z_matmuls((ztA, ztB), t, g)
                if pending is not None:
                    flush_pending()
                if g == 0:
                    nc.vector.reduce_max(mxc[:, 2 * t:2 * t + 1], ztA[:],
                                         mybir.AxisListType.X)
                    nc.vector.reduce_max(mxc[:, 2 * t + 1:2 * t + 2], ztB[:],
                                         mybir.AxisListType.X)
                probA = ppool.tile([128, 2 * PW], b16, name="probA",
                                   tag="prob")
                probB = ppool.tile([128, 2 * PW], b16, name="probB",
                                   tag="prob")
                nc.scalar.activation(probA[:], ztA[:], exp_f,
                                     bias=cb[:, g:g + 1])
                nc.scalar.activation(probB[:], ztB[:], exp_f,
                                     bias=cb[:, g:g + 1])
                pending = (probA, probB, g, t)
                if idx == TQ + 1:
                    emit_collective()
            flush_pending()

            nc.scalar.activation(sig[:], stash[:],
                                 mybir.ActivationFunctionType.Sigmoid,
                                 scale=inv[0:6 * TQ, 0:1])
            nc.sync.dma_start(img, sig[:])
            nc.sync.dma_start(omax, gmx[0:1, 0:1])

    nc.compile()
    return nc


def _get_nc():
    if "nc" not in _CACHE:
        _CACHE["nc"] = _build()
    return _CACHE["nc"]


def _params(mean, alpha, scale, theta, rgb):
    """Per-gaussian z-polynomial coefficients (float64 host math)."""
    ta = 2.0 * np.pi * theta[:, 0].astype(np.float64)
    c, s = np.cos(ta), np.sin(ta)
    sx2 = scale[:, 0].astype(np.float64) ** 2
    sy2 = scale[:, 1].astype(np.float64) ** 2
    a = c * c * sx2 + s * s * sy2
    b = c * s * (sx2 - sy2)
    d = s * s * sx2 + c * c * sy2
    det = a * d - b * b
    inv00 = d / det
    inv11 = a / det
    invc = -2.0 * b / det                 # coefficient of dx*dy in q
    lognorm = -np.log(2.0 * np.pi * np.sqrt(det))
    mx = mean[:, 0, 0].astype(np.float64)
    my = mean[:, 1, 0].astype(np.float64)
    c1 = -0.5 * inv00
    c2 = -0.5 * invc
    c3 = -0.5 * inv11
    c4 = inv00 * mx + 0.5 * invc * my
    c5 = 0.5 * invc * mx + inv11 * my
    c6 = -0.5 * (inv00 * mx * mx + invc * mx * my + inv11 * my * my) + lognorm
    C = np.stack([c1, c2, c3, c4, c5, c6], 0)          # (6, N) float64
    wt = rgb.astype(np.float64) * alpha.astype(np.float64)  # (N, 3)
    norm = np.exp(lognorm)
    return C, wt, norm


def _numpy_kernel(mean, alpha, scale, theta, rgb, pixels):
    """Exact host fallback (used only if a correctness guard fails)."""
    C, wt, norm = _params(mean, alpha, scale, theta, rgb)
    x = pixels[:, :, 0].reshape(-1).astype(np.float64)
    y = pixels[:, :, 1].reshape(-1).astype(np.float64)
    Phi = np.stack([x * x, x * y, y * y, x, y, np.ones_like(x)], 0)
    acc = np.zeros((x.size, 3), dtype=np.float64)
    pmax = 0.0
    for i in range(0, N, 64):
        prob = np.exp(C[:, i:i + 64].T @ Phi)
        pmax = max(pmax, prob.max())
        acc += prob.T @ wt[i:i + 64]
    out = 1.0 / (1.0 + np.exp(-acc / pmax))
    return out.reshape(H, W, 3).astype(np.float32)


def kernel(mean, alpha, scale, theta, rgb, pixels):
    global LAST_RESULTS
    mean = np.asarray(mean, dtype=np.float32)
    alpha = np.asarray(alpha, dtype=np.float32)
    scale = np.asarray(scale, dtype=np.float32)
    theta = np.asarray(theta, dtype=np.float32)
    rgb = np.asarray(rgb, dtype=np.float32)
    pixels = np.asarray(pixels, dtype=np.float32)

    import ml_dtypes
    bf16 = ml_dtypes.bfloat16

    C, wt, norm = _params(mean, alpha, scale, theta, rgb)
    order = np.argsort(-norm, kind="stable")
    norms = norm[order]
    Cs = np.ascontiguousarray(C[0:5, order]).astype(np.float32)
    Ch = Cs.astype(bf16)
    Cl = (Cs.astype(np.float64) - Ch.astype(np.float64)).astype(
        np.float32).astype(bf16)
    cbias = np.ascontiguousarray(
        C[5, order].reshape(G, 128).T).astype(np.float32)      # [128, G]
    wts = np.ascontiguousarray(wt[order]).astype(np.float32)
    nu = float(norms[128])               # largest amplitude outside chunk 0

    # stage row k' = TQ*q + t holds pixel chunk k = Q*t + q of the core slab
    perm = np.array([Q * (kp % TQ) + kp // TQ for kp in range(Q * TQ)])
    in_maps = []
    for c in range(NCORES):
        slab = pixels[ROWS_PER_CORE * c:ROWS_PER_CORE * (c + 1)]  # (64,512,2)
        in_maps.append({
            "xk": np.ascontiguousarray(slab[:, :, 0].reshape(Q * TQ, PW)[perm]),
            "yk": np.ascontiguousarray(slab[:, :, 1].reshape(Q * TQ, PW)[perm]),
            "coef": Cs,
            "coefh": Ch,
            "coefl": Cl,
            "cbias": cbias,
            "wrgb": wts.astype(bf16),
            "dzw": np.zeros((1, 128), dtype=bf16),
            "done": np.ones((1, PW), dtype=bf16),
        })

    from concourse import bass_utils
    nc = _get_nc()
    want_trace = bool(int(os.environ.get("GAUSS_TRACE", "0")))
    if want_trace:
        _ensure_axon_hooks()
    try:
        res = bass_utils.run_bass_kernel_spmd(
            nc, in_maps, core_ids=list(range(NCORES)), trace=want_trace,
        )
    except Exception:
        if not want_trace:
            raise
        res = bass_utils.run_bass_kernel_spmd(
            nc, in_maps, core_ids=list(range(NCORES)), trace=False,
        )
    LAST_RESULTS = res

    out_slabs = []
    for c in range(NCORES):
        a = res.results[c]["img"].reshape(2 * TQ, 3, 2, PW)   # (s, ch, r, w)
        a = np.transpose(a, (0, 2, 3, 1)).reshape(ROWS_PER_CORE, W, 3)
        out_slabs.append(a)
    out = np.concatenate(out_slabs, axis=0)
    m0 = float(res.results[0]["omax"][0, 0])

    ok = np.isfinite(m0) and m0 >= 1.05 * nu and bool(np.all(np.isfinite(out)))
    if ok:
        # split-bf16 z error must be negligible after max-normalization:
        # chunk g's splat weight is <= norms[128g] / m0.
        phimax = max(1.0, float(np.abs(pixels).max()) ** 2)
        delta = {3: 2.0 ** -16, 2: 2.0 ** -8}
        for g in range(1, G):
            sc = float(np.abs(Cs[:, 128 * g:128 * (g + 1)]).sum(0).max())
            zerr = sc * phimax * delta[PASSES[g]]
            img_err = (np.expm1(min(zerr, 30.0))) * norms[128 * g] / m0
            if img_err > 2e-3:
                ok = False
                break
    if not ok:
        return _numpy_kernel(mean, alpha, scale, theta, rgb, pixels)
    return out.astype(np.float32)
